# revision 1
# baseline (speedup 1.0000x reference)
"""GCNConv (rank-1 normalized aggregation) Trainium2 kernel, SPMD over 8 cores.

Math (faithful to the torch/jax reference):
    h    = x @ W
    adj  = symmetric 0/1 adjacency from edge_index (duplicates collapse: SET, not add)
    deg  = adj.sum(1);  dinv = 1/sqrt(deg)
    agg  = dinv @ h                      # rank-1 identity, [F_OUT]
    out  = dinv[:, None] * agg[None, :] + bias

Since agg = (dinv @ x) @ W, h is never materialized:
    v    = dinv @ x            ([F_IN] weighted row-sum, DVE mul + strided reduce)
    agg  = v @ W               (TensorE)
    out_c = dinv_c (x) agg + bias     (rows sharded across cores)

Collectives in this environment have a ~55us fixed latency (measured with a
bare 512B AllReduce), far above the 8-core floor, so instead of sharding the
v-reduction + AllReduce, every core reads the full x (6.1MB, ~17us at HBM BW)
and computes v locally; only the O(N*F_OUT) output is sharded.

The exact deduplicated degree (an integer/sorting problem, not a flops
problem) is computed on host with np.unique; all O(N*F) floating-point work
runs on the NeuronCores.
"""

import numpy as np

N, F_IN, F_OUT = 12000, 128, 256
N_CORES = 8
ROWS = N // N_CORES            # 1500 output rows per core
NT_OUT = 12                    # 12 row tiles per core (padded)
ROWS_PAD = NT_OUT * 128        # 1536
NT_FULL = 96                   # full-x row tiles (padded)
N_PAD = NT_FULL * 128          # 12288
# x rows-per-partition per DMA/compute chunk; small first chunks so DVE
# starts sooner, ramping up once the pipeline is primed
CHUNK_SIZES = [8, 8, 12, 12, 16, 16, 12, 12]
N_CHUNKS = len(CHUNK_SIZES)

_cache = {}


def _build_nc():
    import concourse.bacc as bacc
    import concourse.mybir as mybir
    import concourse.tile as tile

    f32 = mybir.dt.float32
    bf16 = mybir.dt.bfloat16

    nc = bacc.Bacc(
        "TRN2",
        target_bir_lowering=False,
        debug=False,
        num_devices=N_CORES,
    )

    # x and dinvT travel as bf16: halves DMA bytes and DVE mul time; the
    # ~0.3% relative error on v is far inside the 2e-2 gate
    x_d = nc.dram_tensor("x", [N_PAD, F_IN], bf16, kind="ExternalInput")
    # dinvT[p, r] = dinv[p*96 + r] (host-prepared layout matching x view)
    dinvT_d = nc.dram_tensor("dinvT", [128, NT_FULL], bf16, kind="ExternalInput")
    # f32 copy for the ScalarE activation scale operand
    dinvTf_d = nc.dram_tensor("dinvTf", [128, NT_FULL], f32, kind="ExternalInput")
    dinvS_d = nc.dram_tensor("dinvS", [128, NT_OUT], f32, kind="ExternalInput")
    w_d = nc.dram_tensor("weight", [F_IN, F_OUT], bf16, kind="ExternalInput")
    b_d = nc.dram_tensor("bias", [F_OUT], f32, kind="ExternalInput")
    out_d = nc.dram_tensor("out", [ROWS_PAD, F_OUT], f32, kind="ExternalOutput")

    # x view: partition p holds rows [p*96, (p+1)*96) -> one contiguous 48KB
    # read per partition (vs 2048 scattered 512B runs for the (n p) m view)
    x_prm = x_d.ap().rearrange("(p r) m -> p r m", p=128)      # [128,96,128]
    out_pnm = out_d.ap().rearrange("(n p) m -> p n m", p=128)  # [128,12,256]

    dma_engines = [nc.sync, nc.scalar]

    with tile.TileContext(nc) as tc:
        with (
            tc.tile_pool(name="const", bufs=1) as cpool,
            tc.tile_pool(name="xbuf", bufs=1) as xpool,
            tc.tile_pool(name="scl", bufs=3) as spool,
            tc.tile_pool(name="obuf", bufs=1) as opool,
            tc.tile_pool(name="ps", bufs=1, space="PSUM") as psum,
        ):
            # ---- small constants first (cheap), then x chunks ----
            # (keep everything off gpsimd: SWDGE completion latency is ~9us
            # and its drain blocks dependents)
            dinvT = cpool.tile([128, NT_FULL], bf16)
            nc.sync.dma_start(dinvT[:], dinvT_d.ap())
            dinvTf = cpool.tile([128, NT_FULL], f32)
            nc.scalar.dma_start(dinvTf[:], dinvTf_d.ap())
            bias_s = cpool.tile([1, F_OUT], f32)
            nc.scalar.dma_start(bias_s[:], b_d.ap().rearrange("(a n) -> a n", a=1))

            xc = []
            off = 0
            offs = []
            for q in range(N_CHUNKS):
                sz = CHUNK_SIZES[q]
                t = xpool.tile([128, sz, F_IN], bf16, tag=f"xc{q}", name=f"xc{q}")
                dma_engines[q % len(dma_engines)].dma_start(
                    t[:], x_prm[:, off : off + sz, :]
                )
                xc.append(t)
                offs.append(off)
                off += sz

            # needed only mid/late kernel; queue after the x chunks
            dinvS = cpool.tile([128, NT_OUT], f32)
            nc.scalar.dma_start(dinvS[:], dinvS_d.ap())
            w_s = cpool.tile([F_IN, F_OUT], bf16)
            nc.sync.dma_start(w_s[:], w_d.ap())

            ones_col = cpool.tile([128, 1], bf16)
            nc.vector.memset(ones_col[:], 1.0)
            ones_row = cpool.tile([1, 128], f32)
            nc.vector.memset(ones_row[:], 1.0)

            # ---- v = dinv @ x ----
            # per chunk: scaled = x * dinv (DVE); TensorE contracts partitions
            # via ones-matmuls, ALL accumulating into one [1,512] PSUM bank:
            # pvw[0, u] = sum over rows r with r%4 == u//128 of dinv_r*x[r, u%128]
            pvw = psum.tile([1, 512], f32)
            total_sl = sum(CHUNK_SIZES) * F_IN // 512
            sl = 0
            for q in range(N_CHUNKS):
                sz = CHUNK_SIZES[q]
                d_bc = (
                    dinvT[:, offs[q] : offs[q] + sz]
                    .unsqueeze(2)
                    .broadcast_to([128, sz, F_IN])
                )
                scaled = spool.tile([128, sz, F_IN], bf16, tag=f"scaled{q % 3}",
                                    name=f"scaled{q}")
                if q >= N_CHUNKS - 6:
                    # late chunks: split the scaling DVE/ScalarE so the
                    # pipeline tail shortens (ACT does the last 4 rows;
                    # by then the Activation sequencer has issued all DMAs)
                    dv = sz - 4
                    nc.vector.tensor_mul(
                        scaled[:, :dv, :], xc[q][:, :dv, :],
                        d_bc[:, :dv, :],
                    )
                    for r in range(dv, sz):
                        nc.scalar.activation(
                            scaled[:, r, :],
                            xc[q][:, r, :],
                            mybir.ActivationFunctionType.Copy,
                            scale=dinvTf[:, offs[q] + r : offs[q] + r + 1],
                        )
                else:
                    nc.vector.tensor_mul(scaled[:], xc[q][:], d_bc)
                flat = scaled[:].rearrange("p t j -> p (t j)")
                for s in range((sz * F_IN) // 512):
                    nc.tensor.matmul(
                        pvw[:],
                        ones_col[:],
                        flat[:, s * 512 : (s + 1) * 512],
                        start=(sl == 0),
                        stop=(sl == total_sl - 1),
                        skip_group_check=True,
                    )
                    sl += 1
            # fold the 4 t-mod groups: one small strided reduce
            vrow = cpool.tile([1, F_IN], f32)
            nc.vector.tensor_reduce(
                vrow[:],
                pvw[:].rearrange("a (t j) -> a j t", j=F_IN),
                axis=mybir.AxisListType.X,
                op=mybir.AluOpType.add,
            )

            # v [1,128] -> vcol [128,1] via TensorE transpose; cast to bf16
            # (for the A2 matmul whose rhs W is bf16) in the PSUM->SBUF copy
            pvcol = psum.tile([F_IN, 1], f32)
            nc.tensor.transpose(pvcol[:], vrow[:], ones_row[:1, :1])
            vcol = cpool.tile([F_IN, 1], bf16)
            nc.vector.tensor_copy(vcol[:], pvcol[:])

            # ---- A2[p, o] = agg[o] = sum_j v[j] W[j, o]  (v bcast as lhsT) ----
            pA2 = psum.tile([128, F_OUT], f32)
            nc.tensor.matmul(
                pA2[:],
                vcol[:].broadcast_to([F_IN, 128]),
                w_s[:],
                start=True,
                stop=True,
            )
            A2 = cpool.tile([128, F_OUT], f32)
            nc.vector.tensor_copy(A2[:], pA2[:])
            pB2 = psum.tile([128, F_OUT], f32)
            nc.tensor.matmul(pB2[:], ones_row[:], bias_s[:], start=True, stop=True)
            B2 = cpool.tile([128, F_OUT], f32)
            nc.vector.tensor_copy(B2[:], pB2[:])

            # ---- out tile i = (A2 * dinvS_i) + B2, one fused DVE op each ----
            # shrinking DMA groups so the last transfer is small
            out_engines = [nc.sync, nc.scalar]
            og_sizes = [3, 3, 2, 2, 1, 1]
            base = 0
            for g, gsz in enumerate(og_sizes):
                og = opool.tile([128, gsz, F_OUT], f32, tag=f"og{g}",
                                name=f"og{g}")
                for j in range(gsz):
                    i = base + j
                    nc.vector.scalar_tensor_tensor(
                        og[:, j, :],
                        A2[:],
                        dinvS[:, i : i + 1],
                        B2[:],
                        op0=mybir.AluOpType.mult,
                        op1=mybir.AluOpType.add,
                    )
                out_engines[g % 2].dma_start(
                    out_pnm[:, base : base + gsz, :], og[:]
                )
                base += gsz

    nc.compile()
    return nc


def _get_nc():
    if "nc" not in _cache:
        _cache["nc"] = _build_nc()
    return _cache["nc"]


def _host_dinv(edge_index: np.ndarray) -> np.ndarray:
    """Exact deduplicated symmetric degree -> 1/sqrt(deg), matching
    adj[a,b]=1; adj[b,a]=1; deg=adj.sum(1)."""
    a = edge_index[0].astype(np.int64)
    b = edge_index[1].astype(np.int64)
    keys = np.unique(np.concatenate([a * N + b, b * N + a]))
    deg = np.bincount(keys // N, minlength=N).astype(np.float32)
    with np.errstate(divide="ignore"):
        dinv = (np.float32(1.0) / np.sqrt(deg)).astype(np.float32)
    return dinv


def kernel(x, edge_index, weight, bias, _trace=False):
    from concourse import bass_utils

    x = np.ascontiguousarray(x, dtype=np.float32)
    weight = np.ascontiguousarray(weight, dtype=np.float32)
    bias = np.ascontiguousarray(bias, dtype=np.float32)
    dinv = _host_dinv(np.asarray(edge_index))

    nc = _get_nc()

    import ml_dtypes

    bf16 = ml_dtypes.bfloat16
    xp = np.zeros((N_PAD, F_IN), bf16)
    xp[:N] = x.astype(bf16)
    dp = np.zeros((N_PAD,), np.float32)
    dp[:N] = dinv
    # dinvT[p, r] = dinv[p*96 + r], matching the x view "(p r) m -> p r m"
    dinvTf = np.ascontiguousarray(dp.reshape(128, NT_FULL))
    dinvT = dinvTf.astype(bf16)

    w16 = weight.astype(bf16)
    in_maps = []
    for c in range(N_CORES):
        r0 = c * ROWS
        ds = np.zeros((ROWS_PAD,), np.float32)
        ds[:ROWS] = dinv[r0 : r0 + ROWS]
        dinvS = np.ascontiguousarray(ds.reshape(NT_OUT, 128).T)  # [128, 12]
        in_maps.append(
            {
                "x": xp,
                "dinvT": dinvT,
                "dinvTf": dinvTf,
                "dinvS": dinvS,
                "weight": w16,
                "bias": bias,
            }
        )

    res = bass_utils.run_bass_kernel_spmd(
        nc, in_maps, core_ids=list(range(N_CORES)), trace=_trace
    )
    out = np.concatenate(
        [res.results[c]["out"][:ROWS] for c in range(N_CORES)], axis=0
    )
    if _trace:
        _cache["last_results"] = res
    return out



# revision 2
# speedup vs baseline: 1.0576x; 1.0576x over previous
"""GCNConv (rank-1 normalized aggregation) Trainium2 kernel, SPMD over 8 cores.

Math (faithful to the torch/jax reference):
    h    = x @ W
    adj  = symmetric 0/1 adjacency from edge_index (duplicates collapse: SET, not add)
    deg  = adj.sum(1);  dinv = 1/sqrt(deg)
    agg  = dinv @ h = (dinv @ x) @ W        # rank-1 identity, [F_OUT]
    out  = dinv[:, None] * agg[None, :] + bias

v3 design (per core; every core reads the full x, output rows are sharded):
  - x scan: 96 TensorE matmuls with the x row-slice [128,128] as the
    STATIONARY operand (bf16 -> fast weight load) and the dinv column as
    the moving operand: out accumulates v directly as a [128,1] PSUM
    column -- no fold, no transpose, no DVE in the stream.
  - x chunks front-loaded ([32,16,...,4] row-slots) so the last chunk is
    tiny: fewer DMA completion boundaries and a short tail.
  - tail: cast v to bf16 -> one agg matmul -> cast agg into the rhs slot
    (two halves, DVE + ScalarE in parallel) -> 12 out tiles as outer
    products, each split into 4 col-group sub-matmuls (tile_position
    rotation) that execute concurrently -> bf16 copies (DVE/ScalarE
    alternating) -> 4 DMA groups [2,4,4,2] on 2 queues.
  - out is written bf16 and upcast to f32 on host (halves the out drain).
  - PE warmed with discarded matmuls in the pre-stream idle window (HAM).

The exact deduplicated degree (an integer/sorting problem, not a flops
problem) is computed on host with np.unique; all O(N*F) floating-point work
runs on the NeuronCores.
"""

import numpy as np

N, F_IN, F_OUT = 12000, 128, 256
N_CORES = 8
ROWS = N // N_CORES            # 1500 output rows per core
NT_OUT = 12                    # 12 row tiles per core (padded)
ROWS_PAD = NT_OUT * 128        # 1536
NT_FULL = 96                   # full-x row slots per partition
N_PAD = NT_FULL * 128          # 12288
CHUNK_SIZES = [32, 16, 16, 12, 8, 8, 4]   # r-slots per DMA chunk
N_WARM = 12

_cache = {}


def _build_nc():
    import concourse.bacc as bacc
    import concourse.mybir as mybir
    import concourse.tile as tile

    f32 = mybir.dt.float32
    bf16 = mybir.dt.bfloat16

    nc = bacc.Bacc(
        "TRN2",
        target_bir_lowering=False,
        debug=False,
        num_devices=N_CORES,
    )

    x_d = nc.dram_tensor("x", [N_PAD, F_IN], bf16, kind="ExternalInput")
    # cA = [dinvT | W]: dinvT[p, r] = dinv[p*96+r]
    cA_d = nc.dram_tensor("cA", [128, NT_FULL + F_OUT], bf16, kind="ExternalInput")
    # cB row0 = [dinv shard (1536) | agg placeholder (256)]
    #    row1 = [ones (1536)       | bias (256)]
    cB_d = nc.dram_tensor("cB", [2, ROWS_PAD + F_OUT], bf16, kind="ExternalInput")
    out_d = nc.dram_tensor("out", [ROWS_PAD, F_OUT], bf16, kind="ExternalOutput")

    x_prm = x_d.ap().rearrange("(p r) m -> p r m", p=128)      # [128,96,128]
    out_pnm = out_d.ap().rearrange("(n p) m -> p n m", p=128)  # [128,12,256]

    with tile.TileContext(nc) as tc:
        with (
            tc.tile_pool(name="const", bufs=1) as cpool,
            tc.tile_pool(name="xbuf", bufs=1) as xpool,
            tc.tile_pool(name="obuf", bufs=1) as opool,
            tc.tile_pool(name="pc", bufs=1, space="PSUM") as pcpool,
            tc.tile_pool(name="pa", bufs=1, space="PSUM") as papool,
            tc.tile_pool(name="po", bufs=4, space="PSUM") as popool,
        ):
            # ---- const DMAs on the scalar queue ----
            cA = cpool.tile([128, NT_FULL + F_OUT], bf16)
            nc.scalar.dma_start(cA[:], cA_d.ap())
            cB = cpool.tile([2, ROWS_PAD + F_OUT], bf16)
            nc.scalar.dma_start(cB[:], cB_d.ap())

            # ---- x chunk DMAs, all on the sync queue ----
            xc = []
            off = 0
            for q, sz in enumerate(CHUNK_SIZES):
                t = xpool.tile([128, sz, F_IN], bf16, tag=f"xc{q}", name=f"xc{q}")
                nc.sync.dma_start(t[:], x_prm[:, off : off + sz, :])
                xc.append(t)
                off += sz

            # ---- small SBUF consts (DVE memsets, run early) ----
            wcol = cpool.tile([128, 1], bf16)
            nc.vector.memset(wcol[:], 0.0)
            wrow = cpool.tile([128, F_IN], bf16)
            nc.vector.memset(wrow[:], 0.0)

            pvcol = pcpool.tile([128, 1], f32, tag="pvc", name="pvcol")
            pagg = papool.tile([1, F_OUT], f32, tag="pagg", name="pagg")

            # ---- PE warmup: discarded by the scan's start=True ----
            for i in range(N_WARM):
                nc.tensor.matmul(
                    pvcol[:], wrow[:], wcol[:],
                    start=True, stop=True, skip_group_check=True,
                )

            # ---- x scan: 96 matmuls, x slice stationary, accumulate v col
            rg = 0
            for q, sz in enumerate(CHUNK_SIZES):
                for rl in range(sz):
                    nc.tensor.matmul(
                        pvcol[:],
                        xc[q][:, rl, :],
                        cA[:, rg : rg + 1],
                        start=(rg == 0),
                        stop=(rg == NT_FULL - 1),
                        skip_group_check=True,
                    )
                    rg += 1

            # ---- tail: v -> agg -> rhs slot ----
            vcol = cpool.tile([128, 1], bf16)
            nc.vector.tensor_copy(vcol[:], pvcol[:])
            nc.tensor.matmul(
                pagg[:], vcol[:], cA[:, NT_FULL : NT_FULL + F_OUT],
                start=True, stop=True, skip_group_check=True,
            )
            # agg -> cB row0 tail slot, two halves in parallel
            HF = F_OUT // 2
            nc.vector.tensor_copy(cB[0:1, ROWS_PAD : ROWS_PAD + HF], pagg[:, :HF])
            nc.scalar.activation(
                cB[0:1, ROWS_PAD + HF : ROWS_PAD + F_OUT], pagg[:, HF:],
                mybir.ActivationFunctionType.Copy,
            )

            # ---- out tiles: outer products, 4 col-group sub-MMs each ----
            og_tiles = [2, 4, 4, 2]
            og_engines = [nc.sync, nc.scalar, nc.sync, nc.scalar]
            copy_engines = ["v", "s", "v", "s", "v", "s"]
            rhs = cB[:, ROWS_PAD : ROWS_PAD + F_OUT]
            base = 0
            ci = 0
            for g, gsz in enumerate(og_tiles):
                og = opool.tile([128, gsz, F_OUT], bf16, tag=f"og{g}",
                                name=f"og{g}")
                for h in range(gsz // 2):
                    po = popool.tile([128, 2, F_OUT], f32, tag="po",
                                     name=f"po{ci}")
                    for t2 in range(2):
                        n = base + 2 * h + t2
                        for k in range(4):
                            nc.tensor.matmul(
                                po[32 * k : 32 * k + 32, t2, :],
                                cB[:, n * 128 + 32 * k : n * 128 + 32 * k + 32],
                                rhs,
                                start=True, stop=True, skip_group_check=True,
                                tile_position=(0, 32 * k),
                            )
                    dst = og[:, 2 * h : 2 * h + 2, :]
                    if copy_engines[ci] == "v":
                        nc.vector.tensor_copy(dst, po[:])
                    else:
                        nc.scalar.activation(
                            dst, po[:], mybir.ActivationFunctionType.Copy
                        )
                    ci += 1
                og_engines[g].dma_start(out_pnm[:, base : base + gsz, :], og[:])
                base += gsz

    nc.compile()
    return nc


def _get_nc():
    if "nc" not in _cache:
        _cache["nc"] = _build_nc()
    return _cache["nc"]


def _host_dinv(edge_index: np.ndarray) -> np.ndarray:
    """Exact deduplicated symmetric degree -> 1/sqrt(deg), matching
    adj[a,b]=1; adj[b,a]=1; deg=adj.sum(1)."""
    a = edge_index[0].astype(np.int64)
    b = edge_index[1].astype(np.int64)
    keys = np.unique(np.concatenate([a * N + b, b * N + a]))
    deg = np.bincount(keys // N, minlength=N).astype(np.float32)
    with np.errstate(divide="ignore"):
        dinv = (np.float32(1.0) / np.sqrt(deg)).astype(np.float32)
    return dinv


def kernel(x, edge_index, weight, bias, _trace=False):
    from concourse import bass_utils
    import ml_dtypes

    bf16 = ml_dtypes.bfloat16

    x = np.ascontiguousarray(x, dtype=np.float32)
    weight = np.ascontiguousarray(weight, dtype=np.float32)
    bias = np.ascontiguousarray(bias, dtype=np.float32)
    dinv = _host_dinv(np.asarray(edge_index))

    nc = _get_nc()

    xp = np.zeros((N_PAD, F_IN), bf16)
    xp[:N] = x.astype(bf16)
    dp = np.zeros((N_PAD,), np.float32)
    dp[:N] = dinv

    cA = np.ascontiguousarray(
        np.concatenate(
            [dp.reshape(128, NT_FULL).astype(bf16), weight.astype(bf16)], axis=1
        )
    )

    in_maps = []
    for c in range(N_CORES):
        r0 = c * ROWS
        cB = np.zeros((2, ROWS_PAD + F_OUT), bf16)
        cB[0, :ROWS] = dinv[r0 : r0 + ROWS].astype(bf16)
        cB[1, :ROWS_PAD] = bf16(1.0)
        cB[1, ROWS_PAD:] = bias.astype(bf16)
        in_maps.append({"x": xp, "cA": cA, "cB": cB})

    res = bass_utils.run_bass_kernel_spmd(
        nc, in_maps, core_ids=list(range(N_CORES)), trace=_trace
    )
    out = np.concatenate(
        [np.asarray(res.results[c]["out"][:ROWS]) for c in range(N_CORES)],
        axis=0,
    ).astype(np.float32)
    if _trace:
        _cache["last_results"] = res
    return out


# revision 3
# speedup vs baseline: 1.0769x; 1.0183x over previous
"""GCNConv (rank-1 normalized aggregation) Trainium2 kernel, SPMD over 8 cores.

Math (faithful to the torch/jax reference):
    h    = x @ W
    adj  = symmetric 0/1 adjacency from edge_index (duplicates collapse: SET, not add)
    deg  = adj.sum(1);  dinv = 1/sqrt(deg)
    agg  = dinv @ h = (dinv @ x) @ W        # rank-1 identity, [F_OUT]
    out  = dinv[:, None] * agg[None, :] + bias

v5 design (per core; every core reads the full x, output rows are sharded):
  - x scan: 96 TensorE matmuls with the x row-slice [128,128] as the
    STATIONARY operand (bf16 fast weight load) and the dinv column moving:
    v accumulates directly as a [128,1] PSUM column -- no fold/transpose.
  - x chunks front-loaded ([32,16,...,4]) so the last chunk is tiny.
  - tail: cast v to bf16 -> one broadcast agg matmul (agg on all 128
    partitions) -> one cast to bf16 -> each out tile is a single DVE
    tensor_scalar multiply (or ScalarE activation) writing bf16 SBUF
    directly -- no PSUM round-trip, no per-tile matmul+copy.
  - out written bf16 in 3 DMA groups on 2 queues; host upcasts to f32.
  - bias is zero in this workload; a general-bias variant (DVE
    scalar_tensor_tensor with a ones x bias tile) compiles lazily if a
    nonzero bias ever shows up.
  - PE warmed with discarded matmuls in the pre-stream idle window (HAM).

The exact deduplicated degree (an integer/sorting problem, not a flops
problem) is computed on host with np.unique; all O(N*F) floating-point work
runs on the NeuronCores.
"""

import numpy as np

N, F_IN, F_OUT = 12000, 128, 256
N_CORES = 8
ROWS = N // N_CORES            # 1500 output rows per core
NT_OUT = 12                    # 12 row tiles per core (padded)
ROWS_PAD = NT_OUT * 128        # 1536
NT_FULL = 96                   # full-x row slots per partition
N_PAD = NT_FULL * 128          # 12288
CHUNK_SIZES = [32, 16, 16, 12, 8, 8, 4]   # r-slots per DMA chunk
N_WARM = 12

_cache = {}


def _build_nc(with_bias: bool):
    import concourse.bacc as bacc
    import concourse.mybir as mybir
    import concourse.tile as tile

    f32 = mybir.dt.float32
    bf16 = mybir.dt.bfloat16

    nc = bacc.Bacc(
        "TRN2",
        target_bir_lowering=False,
        debug=False,
        num_devices=N_CORES,
    )

    x_d = nc.dram_tensor("x", [N_PAD, F_IN], bf16, kind="ExternalInput")
    # cA = [dinvT | W]: dinvT[p, r] = dinv[p*96+r]
    cA_d = nc.dram_tensor("cA", [128, NT_FULL + F_OUT], bf16, kind="ExternalInput")
    # cC[p, n] = dinv[core_row0 + n*128 + p] (per-tile scale columns)
    cC_d = nc.dram_tensor("cC", [128, NT_OUT], f32, kind="ExternalInput")
    if with_bias:
        bias_d = nc.dram_tensor("biasR", [1, F_OUT], bf16, kind="ExternalInput")
    out_d = nc.dram_tensor("out", [ROWS_PAD, F_OUT], bf16, kind="ExternalOutput")

    x_prm = x_d.ap().rearrange("(p r) m -> p r m", p=128)      # [128,96,128]
    out_pnm = out_d.ap().rearrange("(n p) m -> p n m", p=128)  # [128,12,256]

    with tile.TileContext(nc) as tc:
        with (
            tc.tile_pool(name="const", bufs=1) as cpool,
            tc.tile_pool(name="xbuf", bufs=1) as xpool,
            tc.tile_pool(name="obuf", bufs=1) as opool,
            tc.tile_pool(name="pc", bufs=1, space="PSUM") as pcpool,
            tc.tile_pool(name="pa", bufs=1, space="PSUM") as papool,
        ):
            # ---- const DMAs on the scalar queue ----
            cA = cpool.tile([128, NT_FULL + F_OUT], bf16)
            nc.scalar.dma_start(cA[:], cA_d.ap())
            cC = cpool.tile([128, NT_OUT], f32)
            nc.scalar.dma_start(cC[:], cC_d.ap())
            if with_bias:
                biasR = cpool.tile([1, F_OUT], bf16)
                nc.scalar.dma_start(biasR[:], bias_d.ap())

            # ---- x chunk DMAs, all on the sync queue ----
            xc = []
            off = 0
            for q, sz in enumerate(CHUNK_SIZES):
                t = xpool.tile([128, sz, F_IN], bf16, tag=f"xc{q}", name=f"xc{q}")
                nc.sync.dma_start(t[:], x_prm[:, off : off + sz, :])
                xc.append(t)
                off += sz

            # ---- small SBUF consts (DVE memsets, run early) ----
            wcol = cpool.tile([128, 1], bf16)
            nc.vector.memset(wcol[:], 0.0)
            wrow = cpool.tile([128, F_IN], bf16)
            nc.vector.memset(wrow[:], 0.0)

            pvcol = pcpool.tile([128, 1], f32, tag="pvc", name="pvcol")
            pA2 = papool.tile([128, F_OUT], f32, tag="pA2", name="pA2")

            if with_bias:
                onesrow = cpool.tile([1, 128], bf16)
                nc.vector.memset(onesrow[:], 1.0)
                pB2 = pcpool.tile([128, F_OUT], f32, tag="pB2", name="pB2")
                nc.tensor.matmul(
                    pB2[:], onesrow[:], biasR[:],
                    start=True, stop=True, skip_group_check=True,
                )
                B2 = cpool.tile([128, F_OUT], bf16)
                nc.vector.tensor_copy(B2[:], pB2[:])

            # ---- PE warmup: discarded by the scan's start=True ----
            for i in range(N_WARM):
                nc.tensor.matmul(
                    pvcol[:], wrow[:], wcol[:],
                    start=True, stop=True, skip_group_check=True,
                )

            # ---- x scan: 96 matmuls, x slice stationary, accumulate v col
            rg = 0
            for q, sz in enumerate(CHUNK_SIZES):
                for rl in range(sz):
                    nc.tensor.matmul(
                        pvcol[:],
                        xc[q][:, rl, :],
                        cA[:, rg : rg + 1],
                        start=(rg == 0),
                        stop=(rg == NT_FULL - 1),
                        skip_group_check=True,
                    )
                    rg += 1

            # ---- tail: v -> agg broadcast on all partitions -> bf16 ----
            vcol = cpool.tile([128, 1], bf16)
            nc.vector.tensor_copy(vcol[:], pvcol[:])
            nc.tensor.matmul(
                pA2[:], vcol[:].broadcast_to([F_IN, 128]),
                cA[:, NT_FULL : NT_FULL + F_OUT],
                start=True, stop=True, skip_group_check=True,
            )
            A2 = cpool.tile([128, F_OUT], bf16)
            nc.vector.tensor_copy(A2[:], pA2[:])

            # ---- out tiles: one DVE tensor_scalar (or ScalarE act) each ----
            og_tiles = [4, 4, 4]
            og_engines = [nc.sync, nc.scalar, nc.sync]
            base = 0
            for g, gsz in enumerate(og_tiles):
                og = opool.tile([128, gsz, F_OUT], bf16, tag=f"og{g}",
                                name=f"og{g}")
                for j in range(gsz):
                    n = base + j
                    dst = og[:, j, :]
                    if with_bias:
                        nc.vector.scalar_tensor_tensor(
                            dst, A2[:], cC[:, n : n + 1], B2[:],
                            op0=mybir.AluOpType.mult,
                            op1=mybir.AluOpType.add,
                        )
                    elif j == 1:
                        nc.scalar.activation(
                            dst, A2[:], mybir.ActivationFunctionType.Copy,
                            scale=cC[:, n : n + 1],
                        )
                    else:
                        nc.vector.tensor_scalar_mul(dst, A2[:], cC[:, n : n + 1])
                og_engines[g].dma_start(out_pnm[:, base : base + gsz, :], og[:])
                base += gsz

    nc.compile()
    return nc


def _get_nc(with_bias: bool):
    key = f"nc{int(with_bias)}"
    if key not in _cache:
        _cache[key] = _build_nc(with_bias)
    return _cache[key]


def _host_dinv(edge_index: np.ndarray) -> np.ndarray:
    """Exact deduplicated symmetric degree -> 1/sqrt(deg), matching
    adj[a,b]=1; adj[b,a]=1; deg=adj.sum(1)."""
    a = edge_index[0].astype(np.int64)
    b = edge_index[1].astype(np.int64)
    keys = np.unique(np.concatenate([a * N + b, b * N + a]))
    deg = np.bincount(keys // N, minlength=N).astype(np.float32)
    with np.errstate(divide="ignore"):
        dinv = (np.float32(1.0) / np.sqrt(deg)).astype(np.float32)
    return dinv


def kernel(x, edge_index, weight, bias, _trace=False):
    from concourse import bass_utils
    import ml_dtypes

    bf16 = ml_dtypes.bfloat16

    x = np.ascontiguousarray(x, dtype=np.float32)
    weight = np.ascontiguousarray(weight, dtype=np.float32)
    bias = np.ascontiguousarray(bias, dtype=np.float32)
    dinv = _host_dinv(np.asarray(edge_index))

    with_bias = bool(np.any(bias))
    nc = _get_nc(with_bias)

    xp = np.zeros((N_PAD, F_IN), bf16)
    xp[:N] = x.astype(bf16)
    dp = np.zeros((N_PAD,), np.float32)
    dp[:N] = dinv

    cA = np.ascontiguousarray(
        np.concatenate(
            [dp.reshape(128, NT_FULL).astype(bf16), weight.astype(bf16)], axis=1
        )
    )

    in_maps = []
    for c in range(N_CORES):
        r0 = c * ROWS
        ds = np.zeros((ROWS_PAD,), np.float32)
        ds[:ROWS] = dinv[r0 : r0 + ROWS]
        cC = np.ascontiguousarray(ds.reshape(NT_OUT, 128).T)  # [128, 12]
        m = {"x": xp, "cA": cA, "cC": cC}
        if with_bias:
            m["biasR"] = bias.astype(bf16).reshape(1, F_OUT)
        in_maps.append(m)

    res = bass_utils.run_bass_kernel_spmd(
        nc, in_maps, core_ids=list(range(N_CORES)), trace=_trace
    )
    out = np.concatenate(
        [np.asarray(res.results[c]["out"][:ROWS]) for c in range(N_CORES)],
        axis=0,
    ).astype(np.float32)
    if _trace:
        _cache["last_results"] = res
    return out


# revision 4
# speedup vs baseline: 1.0783x; 1.0013x over previous
"""GCNConv (rank-1 normalized aggregation) Trainium2 kernel, SPMD over 8 cores.

Math (faithful to the torch/jax reference):
    h    = x @ W
    adj  = symmetric 0/1 adjacency from edge_index (duplicates collapse: SET, not add)
    deg  = adj.sum(1);  dinv = 1/sqrt(deg)
    agg  = dinv @ h = (dinv @ x) @ W        # rank-1 identity, [F_OUT]
    out  = dinv[:, None] * agg[None, :] + bias

v5 design (per core; every core reads the full x, output rows are sharded):
  - x scan: 96 TensorE matmuls with the x row-slice [128,128] as the
    STATIONARY operand (bf16 fast weight load) and the dinv column moving:
    v accumulates directly as a [128,1] PSUM column -- no fold/transpose.
  - x chunks front-loaded ([32,16,...,4]) so the last chunk is tiny.
  - tail: cast v to bf16 -> one broadcast agg matmul (agg on all 128
    partitions) -> one cast to bf16 -> each out tile is a single DVE
    tensor_scalar multiply (or ScalarE activation) writing bf16 SBUF
    directly -- no PSUM round-trip, no per-tile matmul+copy.
  - out written bf16 in 3 DMA groups on 2 queues; host upcasts to f32.
  - bias is zero in this workload; a general-bias variant (DVE
    scalar_tensor_tensor with a ones x bias tile) compiles lazily if a
    nonzero bias ever shows up.
  - PE warmed with discarded matmuls in the pre-stream idle window (HAM).

The exact deduplicated degree (an integer/sorting problem, not a flops
problem) is computed on host with np.unique; all O(N*F) floating-point work
runs on the NeuronCores.
"""

import numpy as np

N, F_IN, F_OUT = 12000, 128, 256
N_CORES = 8
ROWS = N // N_CORES            # 1500 output rows per core
NT_OUT = 12                    # 12 row tiles per core (padded)
ROWS_PAD = NT_OUT * 128        # 1536
NT_FULL = 96                   # full-x row slots per partition
N_PAD = NT_FULL * 128          # 12288
CHUNK_SIZES = [32, 16, 16, 12, 8, 8, 4]   # r-slots per DMA chunk
N_WARM = 12

_cache = {}


def _build_nc(with_bias: bool):
    import concourse.bacc as bacc
    import concourse.mybir as mybir
    import concourse.tile as tile

    f32 = mybir.dt.float32
    bf16 = mybir.dt.bfloat16

    nc = bacc.Bacc(
        "TRN2",
        target_bir_lowering=False,
        debug=False,
        num_devices=N_CORES,
    )

    x_d = nc.dram_tensor("x", [N_PAD, F_IN], bf16, kind="ExternalInput")
    # cA = [dinvT | W]: dinvT[p, r] = dinv[p*96+r]
    cA_d = nc.dram_tensor("cA", [128, NT_FULL + F_OUT], bf16, kind="ExternalInput")
    # cC[p, n] = dinv[core_row0 + n*128 + p] (per-tile scale columns)
    cC_d = nc.dram_tensor("cC", [128, NT_OUT], f32, kind="ExternalInput")
    if with_bias:
        bias_d = nc.dram_tensor("biasR", [1, F_OUT], bf16, kind="ExternalInput")
    out_d = nc.dram_tensor("out", [ROWS_PAD, F_OUT], bf16, kind="ExternalOutput")

    x_prm = x_d.ap().rearrange("(p r) m -> p r m", p=128)      # [128,96,128]
    out_pnm = out_d.ap().rearrange("(n p) m -> p n m", p=128)  # [128,12,256]

    with tile.TileContext(nc) as tc:
        with (
            tc.tile_pool(name="const", bufs=1) as cpool,
            tc.tile_pool(name="xbuf", bufs=1) as xpool,
            tc.tile_pool(name="obuf", bufs=1) as opool,
            tc.tile_pool(name="pc", bufs=1, space="PSUM") as pcpool,
            tc.tile_pool(name="pa", bufs=1, space="PSUM") as papool,
        ):
            # ---- const DMAs on the scalar queue ----
            cA = cpool.tile([128, NT_FULL + F_OUT], bf16)
            nc.scalar.dma_start(cA[:], cA_d.ap())
            cC = cpool.tile([128, NT_OUT], f32)
            nc.scalar.dma_start(cC[:], cC_d.ap())
            if with_bias:
                biasR = cpool.tile([1, F_OUT], bf16)
                nc.scalar.dma_start(biasR[:], bias_d.ap())

            # ---- x chunk DMAs, all on the sync queue ----
            xc = []
            off = 0
            for q, sz in enumerate(CHUNK_SIZES):
                t = xpool.tile([128, sz, F_IN], bf16, tag=f"xc{q}", name=f"xc{q}")
                nc.sync.dma_start(t[:], x_prm[:, off : off + sz, :])
                xc.append(t)
                off += sz

            # ---- small SBUF consts (DVE memsets, run early) ----
            wcol = cpool.tile([128, 1], bf16)
            nc.vector.memset(wcol[:], 0.0)
            wrow = cpool.tile([128, F_IN], bf16)
            nc.vector.memset(wrow[:], 0.0)

            pvcol = pcpool.tile([128, 1], f32, tag="pvc", name="pvcol")
            pA2 = papool.tile([128, F_OUT], f32, tag="pA2", name="pA2")

            if with_bias:
                onesrow = cpool.tile([1, 128], bf16)
                nc.vector.memset(onesrow[:], 1.0)
                pB2 = pcpool.tile([128, F_OUT], f32, tag="pB2", name="pB2")
                nc.tensor.matmul(
                    pB2[:], onesrow[:], biasR[:],
                    start=True, stop=True, skip_group_check=True,
                )
                B2 = cpool.tile([128, F_OUT], bf16)
                nc.vector.tensor_copy(B2[:], pB2[:])

            # ---- PE warmup: discarded by the scan's start=True ----
            for i in range(N_WARM):
                nc.tensor.matmul(
                    pvcol[:], wrow[:], wcol[:],
                    start=True, stop=True, skip_group_check=True,
                )

            # ---- x scan: 96 matmuls, x slice stationary, accumulate v col
            rg = 0
            for q, sz in enumerate(CHUNK_SIZES):
                for rl in range(sz):
                    nc.tensor.matmul(
                        pvcol[:],
                        xc[q][:, rl, :],
                        cA[:, rg : rg + 1],
                        start=(rg == 0),
                        stop=(rg == NT_FULL - 1),
                        skip_group_check=True,
                    )
                    rg += 1

            # ---- tail: v -> agg broadcast on all partitions -> bf16 ----
            vcol = cpool.tile([128, 1], bf16)
            nc.vector.tensor_copy(vcol[:], pvcol[:])
            nc.tensor.matmul(
                pA2[:], vcol[:].broadcast_to([F_IN, 128]),
                cA[:, NT_FULL : NT_FULL + F_OUT],
                start=True, stop=True, skip_group_check=True,
            )
            A2 = cpool.tile([128, F_OUT], bf16)
            nc.vector.tensor_copy(A2[:], pA2[:])

            # ---- out tiles ----
            # t0 is formed by ScalarE straight from PSUM (no wait on the A2
            # cast) so the first 1-tile DMA group issues early; t5/t6 also go
            # to ScalarE; the rest are DVE tensor_scalar ops on the bf16 A2.
            # og2's issue lives on Scalar to break the issue FIFO on Sync.
            og_tiles = [1, 3, 4, 4]
            og_engines = [nc.sync, nc.sync, nc.scalar, nc.sync]
            scalar_tiles = {0, 5, 6}
            base = 0
            for g, gsz in enumerate(og_tiles):
                og = opool.tile([128, gsz, F_OUT], bf16, tag=f"og{g}",
                                name=f"og{g}")
                for j in range(gsz):
                    n = base + j
                    dst = og[:, j, :]
                    if with_bias:
                        nc.vector.scalar_tensor_tensor(
                            dst, A2[:], cC[:, n : n + 1], B2[:],
                            op0=mybir.AluOpType.mult,
                            op1=mybir.AluOpType.add,
                        )
                    elif n == 0:
                        nc.scalar.activation(
                            dst, pA2[:], mybir.ActivationFunctionType.Copy,
                            scale=cC[:, n : n + 1],
                        )
                    elif n in scalar_tiles:
                        nc.scalar.activation(
                            dst, A2[:], mybir.ActivationFunctionType.Copy,
                            scale=cC[:, n : n + 1],
                        )
                    else:
                        nc.vector.tensor_scalar_mul(dst, A2[:], cC[:, n : n + 1])
                og_engines[g].dma_start(out_pnm[:, base : base + gsz, :], og[:])
                base += gsz

    nc.compile()
    return nc


def _get_nc(with_bias: bool):
    key = f"nc{int(with_bias)}"
    if key not in _cache:
        _cache[key] = _build_nc(with_bias)
    return _cache[key]


def _host_dinv(edge_index: np.ndarray) -> np.ndarray:
    """Exact deduplicated symmetric degree -> 1/sqrt(deg), matching
    adj[a,b]=1; adj[b,a]=1; deg=adj.sum(1)."""
    a = edge_index[0].astype(np.int64)
    b = edge_index[1].astype(np.int64)
    keys = np.unique(np.concatenate([a * N + b, b * N + a]))
    deg = np.bincount(keys // N, minlength=N).astype(np.float32)
    with np.errstate(divide="ignore"):
        dinv = (np.float32(1.0) / np.sqrt(deg)).astype(np.float32)
    return dinv


def kernel(x, edge_index, weight, bias, _trace=False):
    from concourse import bass_utils
    import ml_dtypes

    bf16 = ml_dtypes.bfloat16

    x = np.ascontiguousarray(x, dtype=np.float32)
    weight = np.ascontiguousarray(weight, dtype=np.float32)
    bias = np.ascontiguousarray(bias, dtype=np.float32)
    dinv = _host_dinv(np.asarray(edge_index))

    with_bias = bool(np.any(bias))
    nc = _get_nc(with_bias)

    xp = np.zeros((N_PAD, F_IN), bf16)
    xp[:N] = x.astype(bf16)
    dp = np.zeros((N_PAD,), np.float32)
    dp[:N] = dinv

    cA = np.ascontiguousarray(
        np.concatenate(
            [dp.reshape(128, NT_FULL).astype(bf16), weight.astype(bf16)], axis=1
        )
    )

    in_maps = []
    for c in range(N_CORES):
        r0 = c * ROWS
        ds = np.zeros((ROWS_PAD,), np.float32)
        ds[:ROWS] = dinv[r0 : r0 + ROWS]
        cC = np.ascontiguousarray(ds.reshape(NT_OUT, 128).T)  # [128, 12]
        m = {"x": xp, "cA": cA, "cC": cC}
        if with_bias:
            m["biasR"] = bias.astype(bf16).reshape(1, F_OUT)
        in_maps.append(m)

    res = bass_utils.run_bass_kernel_spmd(
        nc, in_maps, core_ids=list(range(N_CORES)), trace=_trace
    )
    out = np.concatenate(
        [np.asarray(res.results[c]["out"][:ROWS]) for c in range(N_CORES)],
        axis=0,
    ).astype(np.float32)
    if _trace:
        _cache["last_results"] = res
    return out
